# revision 25
# baseline (speedup 1.0000x reference)
"""Multi-head attention block (QKV proj -> per-(n,head) softmax attention over
the a-axis -> output proj) on 8 Trainium2 NeuronCores.

Sharding: data-parallel over the n axis (256 -> 32 per core). Weights are
replicated. No collectives.

Design (HW-calibrated; see git-less history in session notes):
  - all-bf16 operands: x host-pre-transposed to [n, p, c, a] (contiguous 2KB
    partition bursts) and cast bf16; weights host-permuted into device
    layout (one contiguous DMA each); per-partition q/k bias host-computed.
  - q^T/k^T feature-major (lhsT = w_qkv columns, rhs = x^T), batched over
    n-pairs for 512-wide moving operands; bias via DVE tensor_scalar on the
    PSUM->SBUF eviction.
  - v token-major WITHOUT bias: softmax(s) @ (v + bv) = softmax(s)@v + bv,
    so bv routes through the projection bias row (host precomputes
    bias_row = bv @ w_proj + b_proj); v eviction is a plain ACT copy.
  - scores transposed s^T[j,i] = k^T.T @ q^T, head pairs packed into PE row
    groups (tile_position); softmax = pure exp (bounded inputs); ONE
    accumulation group per PSUM bank (each extra group start costs a
    ~185-370ns bank clear on HW).
  - AV fused with the denominator: stationary [v_h | ones64] -> PSUM rows
    0-63 unnormalized out^T, rows 64-127 sum_j p replicated; one 8-matmul
    accumulation group per head-pair covers BOTH slices of the n-pair
    (2-bank psum tile). (Writing the two heads to separate PE column
    groups would halve the normalize work, but tile_position col-offset
    64 silently drops the high-half writes - probed both ways on HW.)
  - 1/l = exp(-ln l), BOTH on ACT from one LUT table set
    (natural_log_exp_and_others has ln+exp+identity+copy -> no 1.28us
    table reloads). The DVE reciprocal is ~8 cycles/element on HW - far
    too slow. Two FD-512 DVE muls per head-pair finish normalization.
  - y = out @ w_proj + bias_row token-major; bias as a K=1 matmul LAST in
    the accumulation group (a dependency-free first matmul would claim its
    PSUM slot early and starve the next pair's qkv groups).
All matmul stationaries are bf16 (FWL weight loads, hidden under N>=256
moving streams - HW-verified 107ns/MM at N=256, 277ns at N=512).
"""

import numpy as np

import concourse.bass as bass
import concourse.mybir as mybir
import concourse.tile as tile

N_CORES = 8
N_TOTAL = 256
A = 256  # tokens per n-slice
DIM = 512
H = 8
DH = 64
N_PER = N_TOTAL // N_CORES  # 32
V_RING = 3  # persistent [v|ones] tiles (ones half initialized once)

F32 = mybir.dt.float32
F32R = mybir.dt.float32r
BF16 = mybir.dt.bfloat16
F16 = mybir.dt.float16


def _patch_tile_drain():
    """The stock TileContext exit emits one SP Drain carrying every
    outstanding semaphore wait; this walrus's CTRL encoding only fits a
    couple of sync-wait commands per instruction, so split the waits across
    a chain of drains (sequential on SP => semantically identical)."""
    from concourse.tile import TileContext, ScopedClock

    if getattr(TileContext, "_drain_split_patched", False):
        return

    def _split_drain_and_barrier(self, tick_clock, wait_clock):
        nc = self.nc
        drain_inst = nc.sync.drain()
        wait_clock.add_sem_waits(
            drain_inst.ins, ScopedClock({None: tick_clock.global_clock})
        )
        si = drain_inst.ins.sync_info
        waits = list(si.on_wait or []) if si is not None else []
        MAX_W = 1
        if len(waits) > MAX_W:
            si.on_wait = waits[:MAX_W]
            rest = waits[MAX_W:]
            while rest:
                chunk, rest = rest[:MAX_W], rest[MAX_W:]
                extra = nc.sync.drain()
                extra.ins.sync_info = mybir.SyncInfo(on_wait=chunk, on_update=[])
        nc.all_engine_barrier()
        assert self.sems is not None
        popped = nc._tile_sem_poison_stack.pop()
        assert popped is self._sem_poison
        nc.clear_and_free_semaphores(list(self.sems.allocated().values()))
        nc.all_engine_barrier()

    TileContext._drain_and_barrier = _split_drain_and_barrier
    TileContext._drain_split_patched = True


def build_bass(n_per: int = N_PER, trace_sim: bool = False, reps: int = 1):
    """Build the per-core Bass program. Inputs (host-prepared):
      x    [n_per, DIM, A]  bf16   (pre-transposed, feature-major)
      wq   [128, 4, 3, DIM] bf16   (c p)(h t d) -> p c t (h d)
      wp   [128, 4, DIM]    bf16   (c p) e -> p c e
      bqk  [128, 8]         f32    per-partition q/k bias (blk 0-3 q, 4-7 k)
      brow [1, DIM]         f32    b_proj + b_qkv_v @ w_proj
    Output: y [n_per, A, DIM] f32.
    reps>1 re-runs the whole n-loop in a dynamic loop (slope timing only)."""
    _patch_tile_drain()
    nc = bass.Bass()

    x_d = nc.dram_tensor("x", [n_per, 128, 4, A], BF16, kind="ExternalInput")
    wq_d = nc.dram_tensor("wq", [128, 4, 3, DIM], BF16, kind="ExternalInput")
    wp_d = nc.dram_tensor("wp", [128, 4, DIM], BF16, kind="ExternalInput")
    bqk_d = nc.dram_tensor("bqk", [128, 8], F32, kind="ExternalInput")
    brow_d = nc.dram_tensor("brow", [1, DIM], F32R, kind="ExternalInput")
    y_d = nc.dram_tensor("y", [n_per, A, DIM], F32, kind="ExternalOutput")

    with tile.TileContext(nc, trace_sim=trace_sim) as tc:
        ctx_lp = nc.allow_low_precision(
            "bf16 intermediates (attention tolerates 8-bit mantissa; "
            "validated ~5e-3 max rel err vs 2e-2 budget)"
        )
        ctx_lp.__enter__()
        with (
            tc.tile_pool(name="consts", bufs=1) as consts,
            tc.tile_pool(name="xt", bufs=6) as p_xt,
            tc.tile_pool(name="qk", bufs=3) as p_qk,
            tc.tile_pool(name="vv", bufs=3) as p_v,
            tc.tile_pool(name="pt", bufs=6) as p_pt,
            tc.tile_pool(name="ot", bufs=3) as p_ot,
            tc.tile_pool(name="li", bufs=6) as p_li,
            tc.tile_pool(name="yy", bufs=4) as p_y,
            tc.tile_pool(name="psA", bufs=2, space="PSUM") as ps_a,
            tc.tile_pool(name="psB", bufs=2, space="PSUM") as ps_b,
            tc.tile_pool(name="psC", bufs=2, space="PSUM") as ps_c,
        ):
            # ---- constants / weights (loaded once, host-prelaid) ----
            wq_sb = consts.tile([128, 4, 3, DIM], BF16, tag="wq")
            nc.sync.dma_start(out=wq_sb, in_=wq_d[:, :, :, :])
            wp_sb = consts.tile([128, 4, DIM], BF16, tag="wp")
            nc.sync.dma_start(out=wp_sb, in_=wp_d[:, :, :])
            bqk_sb = consts.tile([128, 8], F32, tag="bqk")
            nc.sync.dma_start(out=bqk_sb, in_=bqk_d[:, :])
            brow_sb = consts.tile([1, DIM], F32R, tag="brow")
            nc.sync.dma_start(out=brow_sb, in_=brow_d[:, :])

            onesF = consts.tile([128, 1024], F32, tag="onesF")
            nc.vector.memset(onesF, 1.0)
            ones1 = consts.tile([1, 128], F32R, tag="ones1")
            nc.vector.tensor_copy(out=ones1, in_=onesF[0:1, 0:128])

            # persistent [v | ones] ring: cols 0-63 v (rewritten per slice),
            # cols 64-127 ones (written once here)
            v_ring = []
            for r in range(V_RING):
                vt = consts.tile([128, 2, H, 2 * DH], BF16, tag=f"vring{r}",
                                 name=f"vring{r}")
                nc.vector.tensor_copy(
                    out=vt[:, :, :, DH : 2 * DH],
                    in_=onesF.rearrange("p (a b c) -> p a b c", a=2, b=H),
                )
                v_ring.append(vt)

            import contextlib

            rep_ctx = tc.For_i(0, reps, 1) if reps > 1 else contextlib.nullcontext()
            with rep_ctx:
                _emit_main_loop(
                    nc, tc, n_per,
                    dict(p_xt=p_xt, p_qk=p_qk, p_v=p_v, p_pt=p_pt,
                         p_ot=p_ot, p_li=p_li, p_y=p_y, ps_a=ps_a,
                         ps_b=ps_b, ps_c=ps_c),
                    dict(x_d=x_d, y_d=y_d, wq_sb=wq_sb, wp_sb=wp_sb,
                         bqk_sb=bqk_sb, brow_sb=brow_sb, ones1=ones1,
                         v_ring=v_ring),
                )

    _split_excess_waits(nc)
    return nc


def _emit_main_loop(nc, tc, n_per, pools, env):
    p_xt = pools["p_xt"]; p_qk = pools["p_qk"]; p_v = pools["p_v"]
    p_pt = pools["p_pt"]; p_ot = pools["p_ot"]; p_li = pools["p_li"]
    p_y = pools["p_y"]
    ps_a = pools["ps_a"]; ps_b = pools["ps_b"]; ps_c = pools["ps_c"]
    x_d = env["x_d"]; y_d = env["y_d"]; wq_sb = env["wq_sb"]
    wp_sb = env["wp_sb"]; bqk_sb = env["bqk_sb"]; brow_sb = env["brow_sb"]
    ones1 = env["ones1"]; v_ring = env["v_ring"]

    assert n_per % 2 == 0
    for np2 in range(n_per // 2):
        n0 = 2 * np2
        # x^T for the n-pair: [128, kc, nn, 256] bf16
        xT_sb = p_xt.tile([128, 4, 2, A], BF16, tag="xT")
        for nn in range(2):
            nc.sync.dma_start(
                out=xT_sb[:, :, nn, :],
                in_=x_d[n0 + nn],
            )

        # q^T / k^T feature-major for both n: [128, blk, nn, 256] bf16
        qkT_sb = p_qk.tile([128, 8, 2, A], BF16, tag="qkT")
        for blk in range(8):
            t_idx = 0 if blk < 4 else 1
            hp = blk % 4
            qk_ps = ps_a.tile([128, 2, A], F32, tag="psA")
            for kc in range(4):
                nc.tensor.matmul(
                    qk_ps,
                    wq_sb[:, kc, t_idx, hp * 128 : (hp + 1) * 128],
                    xT_sb[:, kc, :, :],
                    start=(kc == 0),
                    stop=(kc == 3),
                )
            # all qkv evictions on DVE: ACT is saturated by exp + ln/exp
            nc.vector.tensor_scalar_add(
                out=qkT_sb[:, blk, :, :],
                in0=qk_ps,
                scalar1=bqk_sb[:, blk : blk + 1],
            )

        # v token-major for BOTH slices, no bias (bv folded into brow)
        vts = []
        for nn in range(2):
            vt = v_ring[(n0 + nn) % V_RING]
            for tb in range(2):
                v_ps = ps_a.tile([128, H, DH], F32, tag="psA")
                for kc in range(4):
                    nc.tensor.matmul(
                        v_ps,
                        xT_sb[:, kc, nn, tb * 128 : (tb + 1) * 128],
                        wq_sb[:, kc, 2, :],
                        start=(kc == 0),
                        stop=(kc == 3),
                    )
                # ACT, not DVE: this copy gates the AV matmuls, and the DVE
                # queue (qkv evictions + normalize muls) head-of-line blocks
                # it - measured +150us when moved to DVE
                nc.scalar.copy(out=vt[:, tb, :, 0:DH], in_=v_ps)
            vts.append(vt)

        # attention, pair-batched: per head-pair one 8-MM AV group into a
        # 2-bank psum tile (nn = bank), one FD-1024 reciprocal, two FD-512
        # normalize muls
        outT_pair = p_ot.tile([128, 4, 2, A], BF16, tag="outT")
        for hp in range(4):
            pT_pair = p_pt.tile([128, 2, 4, A], BF16, tag="pT")
            for nn in range(2):
                for hi in range(2):
                    off = hi * DH
                    # one accumulation group per bank: start clears the
                    # bank, the jb=1 matmul overwrites its (cleared) slot
                    sT_ps = ps_b.tile([128, 2, A], F32, tag="psB")
                    for jb in range(2):
                        nc.tensor.matmul(
                            sT_ps[:, jb, :],
                            qkT_sb[
                                off : off + DH, 4 + hp, nn,
                                jb * 128 : (jb + 1) * 128,
                            ],
                            qkT_sb[off : off + DH, hp, nn, :],
                            start=(jb == 0),
                            stop=(jb == 1),
                            tile_position=(off, 0),
                            skip_group_check=True,
                        )
                    nc.scalar.activation(
                        out=pT_pair[:, nn, hi * 2 : hi * 2 + 2, :],
                        in_=sT_ps,
                        func=mybir.ActivationFunctionType.Exp,
                        scale=0.125,
                    )

            # AV fused with denominator: stationary [v_h | ones64]
            # -> rows 0-63 out^T_unnorm, rows 64-127 l replicated
            av2 = ps_c.tile([128, 2, 2, A], F32, tag="psC")
            for nn in range(2):
                for hi in range(2):
                    h = 2 * hp + hi
                    for jb in range(2):
                        nc.tensor.matmul(
                            av2[:, nn, hi, :],
                            vts[nn][:, jb, h, :],
                            pT_pair[:, nn, hi * 2 + jb, :],
                            start=(hi == 0 and jb == 0),
                            stop=(hi == 1 and jb == 1),
                            skip_group_check=True,
                        )
            # 1/l as exp(-ln l): both on ACT from ONE LUT table set
            # (natural_log_exp_and_others holds ln+exp+identity+copy, so no
            # 1.3us table reloads); the DVE iterative-divide reciprocal runs
            # at ~8 cycles/element on HW and is far too slow here
            lt = p_li.tile([64, 2, 2, A], F16, tag="lt")
            nc.scalar.activation(
                out=lt,
                in_=av2[DH : 2 * DH, :, :, :],
                func=mybir.ActivationFunctionType.Ln,
            )
            rinv = p_li.tile([64, 2, 2, A], F16, tag="Rinv")
            nc.scalar.activation(
                out=rinv,
                in_=lt,
                func=mybir.ActivationFunctionType.Exp,
                scale=-1.0,
            )
            for hi in range(2):
                nc.vector.tensor_mul(
                    out=outT_pair[hi * DH : (hi + 1) * DH, hp, :, :],
                    in0=av2[0:DH, :, hi, :],
                    in1=rinv[:, :, hi, :],
                )

        # y = out @ w_proj + brow (bias matmul FIRST: clears the bank)
        for nn in range(2):
            n = n0 + nn
            y_sb = p_y.tile([128, 2, DIM], F32, tag="y")
            for tb in range(2):
                # bias matmul LAST: the psum slot is only claimed once outT
                # is ready (a dependency-free first matmul would hoard the
                # slot and starve the next pair's qkv groups)
                y_ps = ps_a.tile([128, DIM], F32, tag="psA")
                for fc in range(4):
                    nc.tensor.matmul(
                        y_ps,
                        outT_pair[:, fc, nn, tb * 128 : (tb + 1) * 128],
                        wp_sb[:, fc, :],
                        start=(fc == 0),
                        stop=False,
                    )
                nc.tensor.matmul(
                    y_ps, ones1, brow_sb, start=False, stop=True
                )
                nc.vector.tensor_copy(out=y_sb[:, tb, :], in_=y_ps)
                nc.sync.dma_start(
                    out=y_d[n, tb * 128 : (tb + 1) * 128, :], in_=y_sb[:, tb, :]
                )


_MAX_WAITS = 1


def _split_excess_waits(nc):
    """Walrus's per-instruction sync-wait budget is tiny (observed failures at
    3 waits on both CTRL and the fused-LDWEIGHTS matmul encoding). Move excess
    waits onto same-engine NoOps inserted immediately before the instruction
    (program order on one engine => waits still all honored before it runs)."""
    nonce = 0
    for fn in nc.m.functions:
        for bb in fn.blocks:
            insts = list(bb.instructions)
            out = []
            for inst in insts:
                si = inst.sync_info
                waits = list(si.on_wait) if si is not None and si.on_wait else []
                if len(waits) > _MAX_WAITS:
                    keep = waits[: _MAX_WAITS]
                    rest = waits[_MAX_WAITS:]
                    while rest:
                        chunk, rest = rest[:_MAX_WAITS], rest[_MAX_WAITS:]
                        if inst.engine == mybir.EngineType.Pool:
                            nop = mybir.InstDrain(name=f"I-waitsplit-{nonce}")
                        else:
                            nop = mybir.InstNoOp(name=f"I-waitsplit-{nonce}")
                        nonce += 1
                        nop.engine = inst.engine
                        nop.sync_info = mybir.SyncInfo(on_wait=chunk, on_update=[])
                        nc.register_instruction(nop)
                        out.append(nop)
                    si.on_wait = keep
                out.append(inst)
            if len(out) != len(insts):
                bb.instructions = out


def prepare_in_maps(inputs):
    """Host-side prep: transpose+cast x, permute weights into device layout,
    compute the per-partition q/k bias layout and the fused proj bias row."""
    import ml_dtypes

    bf16 = ml_dtypes.bfloat16

    x = np.asarray(inputs["x"], dtype=np.float32)
    w_qkv = np.asarray(inputs["w_qkv"], dtype=np.float32)
    b_qkv = np.asarray(inputs["b_qkv"], dtype=np.float32)
    w_proj = np.asarray(inputs["w_proj"], dtype=np.float32)
    b_proj = np.asarray(inputs["b_proj"], dtype=np.float32)

    b, n, a, dim = x.shape
    assert (b, n, a, dim) == (1, N_TOTAL, A, DIM)

    # x: [n, a, d] -> [n, p, c, a] bf16 (feature-major, partition-interleaved
    # so each SBUF partition line is one contiguous 2KB burst)
    xs = np.ascontiguousarray(
        x.reshape(N_TOTAL, A, 4, 128)
        .transpose(0, 3, 2, 1)
        .astype(bf16)
    )
    # wq: [512, 1536] -> [p, c, t, (h d)]: row (c p), col (h t d)
    wq_dev = np.ascontiguousarray(
        w_qkv.reshape(4, 128, H, 3, DH)
        .transpose(1, 0, 3, 2, 4)
        .reshape(128, 4, 3, DIM)
        .astype(bf16)
    )
    # wp: [512, 512] -> [p, c, e]
    wp_dev = np.ascontiguousarray(
        w_proj.reshape(4, 128, DIM).transpose(1, 0, 2).astype(bf16)
    )
    # bqk[p, blk]: blk 0-3 = q head-pairs, 4-7 = k; feature f = hp*128+p
    bq3 = b_qkv.reshape(H, 3, DH)  # [h, t, d]
    bqk = np.empty((128, 8), np.float32)
    for blk in range(8):
        t_idx = 0 if blk < 4 else 1
        hp = blk % 4
        f = hp * 128 + np.arange(128)
        bqk[:, blk] = bq3[f // DH, t_idx, f % DH]
    # brow = b_proj + bv @ w_proj
    bv = bq3[:, 2, :].reshape(DIM)
    brow = (
        b_proj.astype(np.float64) + bv.astype(np.float64) @ w_proj.astype(np.float64)
    ).astype(np.float32).reshape(1, DIM)

    return [
        {
            "x": np.ascontiguousarray(xs[c * N_PER : (c + 1) * N_PER]),
            "wq": wq_dev,
            "wp": wp_dev,
            "bqk": bqk,
            "brow": brow,
        }
        for c in range(N_CORES)
    ]


_NC_CACHE = {}


def _get_nc(n_per: int = N_PER):
    if n_per not in _NC_CACHE:
        _NC_CACHE[n_per] = build_bass(n_per)
    return _NC_CACHE[n_per]


def kernel(**inputs) -> np.ndarray:
    from concourse.bass_utils import run_bass_kernel_spmd

    in_maps = prepare_in_maps(inputs)
    nc = _get_nc()
    res = run_bass_kernel_spmd(nc, in_maps, core_ids=list(range(N_CORES)))
    y = np.concatenate([res.results[c]["y"] for c in range(N_CORES)], axis=0)
    return y.reshape(1, N_TOTAL, A, DIM).astype(np.float32)
